# revision 29
# baseline (speedup 1.0000x reference)
"""Trainium2 Bass kernel for nn_CapsuleLayer (conv capsule layer with dynamic routing).

Full (unsharded) inputs in, full output out. Sharding: data-parallel over the
num_capsules axis A=32 -> 8 cores x 4 capsules each (x windows replicated).

v4 design: priors are NEVER materialized (v3's phase A - 152us of FD=256
matmuls, 77us of LDWEIGHTS, 120us of strided ACT copies and 166KB/part of
SBUF - is gone entirely). Per 128-row unit ((a,p) rows):

  s0 (PE): sT0[d, r] = sum_n priors / N via 18 accumulating matmuls per
      segment with wde stationaries (LDW is 16 cols -> ~13ns), then one
      transpose back to p-major.
  per routing iter:
    g-side on PE (VW trick): vT = transpose(v); VW[r, (ch,g,c)] =
        vT^T @ Wg (5 matmuls per segment, shared-weight moving operand);
        then g = sum_c xws * VW: one 2x DVE mul + 3-add tree over c.
        This replaces v3's 16 per-d muls + add tree (the dominant DVE
        and ACT cost at ~10us/unit-iter across engines).
    softmax: tensor_reduce max (negate) -> ACT exp(bias=-max, accum=sum).
    s-side: 3 e-transposes, 18 REGULAR replication matmuls
        (EB[(g,c),ch,r] = e[r,(ch,g)]), one ACT drain, EXT = xwt * EB
        (one 2x DVE mul), 18 accumulating matmuls with wde.
    squash: sqrt(x) = exp(0.5*ln(x)) on the pre-placed natural_log_exp
        table (no table reloads).

Measured (same-session back-to-back): v3 (priors + 16-mul g-side)
524-538us; v4 500us; v4.4 (split VW/EB PSUM pools + s0-ahead) 502us vs
v4's 598us in the adjacent run; v4.5 (+ 3-unit weave, sbig/lg bufs=3)
409us; v4.6 (this file: VW/EB half-drains pipelined with their DVE
consumers and the first 9 s-matmul chunks) 393us. Device timing drifts +-10-20% run-to-run, so only
adjacent-run comparisons are meaningful. Variants that
measured WORSE, kept here for the record: d-major s-matmuls + s0-ahead
(v4.1, 554us - FD=128 matmuls at the cold 1.2GHz clock beat FD=16 only
when HAM warms, and it never does: PE busy comes in sub-3.4us bursts, so
the clock gate never opens); broadcast-AP LDWEIGHTS to fold the c-replication
into transposes (v4.2, fails BIR verification: "RHS AP can only have one
free dimension"); v3's EX-transpose s-side instead of replication matmuls
(v4.3, 569us - 18 transpose-mode ops/unit-iter cost more PE slice time
than 18 regular matmuls plus the eT prologue, and push 2 drains to DVE).
"""
import os
import numpy as np

import concourse.bass as bass
import concourse.bacc as bacc
import concourse.mybir as mybir
import concourse.tile as tile
from concourse.bass_utils import run_bass_kernel_spmd

# problem constants (hardcoded per contract)
K = 3
B, Ci, H, Wd, Cin = 4, 32, 14, 14, 8
A, N, D = 32, 288, 16
w = 12
P = B * w * w           # 576 positions
G = 16                  # route nodes per chunk
CH = N // G             # 18 chunks; G*Cin = 128 = contraction per chunk
AA = A // 8             # capsules per core
NU = (AA * P) // 128    # 18 units of 128 (a,p) rows
HC = CH // 2            # 9 chunks per PSUM half
HW_ = HC * G * Cin      # 1152 columns per half

F32 = mybir.dt.float32
F16 = mybir.dt.float16
AL = mybir.AluOpType
AF = mybir.ActivationFunctionType
AX = mybir.AxisListType

LAST_RESULT = None

_prog_cache = {}


def _slab_slot(u):
    """xws slab slot for unit u: rows of unit u are xw[p(u,r)] with
    p(u,r) = (128*u + r) mod 576; 9 precomputed alignments cover all units."""
    um = u % 9
    if um <= 3:
        return um
    if um == 4:
        return 8
    return 4 + (um - 5)


def _segments(u):
    """Unit u covers flattened (a,p) rows [128u, 128u+128).
    Returns [(a, p0, rowofs, cnt)]; ro is always 0 or 64."""
    segs = []
    r = u * 128
    end = r + 128
    while r < end:
        a = r // P
        p0 = r % P
        cnt = min(end - r, P - p0)
        segs.append((a, p0, r - u * 128, cnt))
        r += cnt
    return segs


def _build_program():
    key = ("v4",)
    if key in _prog_cache:
        return _prog_cache[key]

    nc = bacc.Bacc()
    # xwt[(g,c), ch, p] : xw transposed to the (g,c)-partition domain
    xwt_d = nc.dram_tensor("xwt", [128, CH, P], F16, kind="ExternalInput")
    # xws2[p-row, slab, ch, g, c] : xw in p-partition rows at 9 alignments
    xws2_d = nc.dram_tensor("xws2", [128, 9, CH, G, Cin], F16,
                            kind="ExternalInput")
    # wgd[d, a, (ch,g,c)] : VW moving operand (contraction over d=16)
    wgd_d = nc.dram_tensor("wgd", [16, AA, CH * G * Cin], F16,
                           kind="ExternalInput")
    # wde[(g,c), a, ch, d] : stationary for s0 chain AND moving for s-matmuls
    wde_d = nc.dram_tensor("wde", [128, AA, CH, D], F16, kind="ExternalInput")
    # repg[q=(ch6,g), ch%6, (g,c)] : replication stationaries
    repg_d = nc.dram_tensor("repg", [96, 6, 128], F16, kind="ExternalInput")
    bunit_d = nc.dram_tensor("bunit", [128, NU, D], F32, kind="ExternalInput")
    out_d = nc.dram_tensor("out", [AA, P, D], F32, kind="ExternalOutput")

    VWC = CH * G * Cin      # 2304 VW/EB columns
    BIGC = 2560             # padded to 5 PSUM banks (f32)

    with tile.TileContext(nc) as tc:
        with (
            tc.tile_pool(name="const", bufs=1) as cp,
            tc.tile_pool(name="sbig", bufs=3) as tp,
            tc.tile_pool(name="lg", bufs=3) as lp,
            tc.tile_pool(name="sm", bufs=3) as sp,
            tc.tile_pool(name="psum_vw", bufs=1, space="PSUM") as qv,
            tc.tile_pool(name="psum_eb", bufs=1, space="PSUM") as qe,
            tc.tile_pool(name="psum_sv", bufs=1, space="PSUM") as qs,
            tc.tile_pool(name="psum_xp", bufs=1, space="PSUM") as qx,
        ):
            # Pre-place the one ACT table covering copy/exp/ln so the
            # lowering pass never swaps tables (1283ns per reload).
            nc.scalar.add_instruction(mybir.InstLoadActFuncSet(
                name=nc.get_next_instruction_name(),
                act_func_set_id=6,  # natural_log_exp_and_others
                ins=[], outs=[]))

            # ---- input DMAs in first-use order: wde+xwt gate s0 of unit 0;
            # wgd/xws2/repg are first needed when iter-1 routing starts.
            wde = cp.tile([128, AA, CH, D], F16)
            nc.sync.dma_start(wde[:], wde_d[:])
            xwt = cp.tile([128, CH, P], F16)
            nc.sync.dma_start(xwt[:], xwt_d[:])
            bunit = cp.tile([128, NU, D], F32)
            nc.sync.dma_start(bunit[:], bunit_d[:])
            wgs = cp.tile([16, AA, CH * G * Cin], F16)
            nc.sync.dma_start(wgs[:], wgd_d[:])
            repg = cp.tile([96, 6, 128], F16)
            nc.sync.dma_start(repg[:], repg_d[:])
            from concourse.masks import make_identity
            ident = cp.tile([128, 128], F16)
            make_identity(nc, ident[:])
            xws2 = cp.tile([128, 9, CH, G, Cin], F16)
            for sl in (0, 1, 2, 3, 8, 4, 5, 6, 7):
                nc.sync.dma_start(xws2[:, sl], xws2_d[:, sl])

            def squash_pre(s, sq):
                junk = sp.tile([128, D], F32, tag="sqjunk")
                sn = sp.tile([128, 1], F32, tag="sn" + sq)
                nc.vector.scalar_tensor_tensor(
                    out=junk[:], in0=s[:], scalar=1.0, in1=s[:],
                    op0=AL.mult, op1=AL.mult, accum_out=sn[:])
                u1 = sp.tile([128, 1], F32, tag="u1")
                nc.vector.tensor_scalar_add(u1[:], sn[:], 1.0)
                r = sp.tile([128, 1], F32, tag="r" + sq)
                nc.vector.reciprocal(r[:], u1[:])
                return sn, r

            def squash_act(sn, sq):
                t = sp.tile([128, 1], F32, tag="t")
                nc.scalar.activation(t[:], sn[:], AF.Ln)
                rt = sp.tile([128, 1], F32, tag="rt" + sq)
                nc.scalar.activation(rt[:], t[:], AF.Exp, scale=0.5)
                return rt

            def squash_post(s, rt, r, sq, dt):
                f = sp.tile([128, 1], F32, tag="f")
                nc.gpsimd.tensor_mul(f[:], rt[:], r[:])
                o = sp.tile([128, D], dt, tag="o" + sq)
                nc.vector.tensor_scalar_mul(o[:], s[:], f[:])
                return o

            s0_done = {}

            def s0_gen(u):
                """s0 back-transposed to p-major SBUF, emitted 2 units ahead
                so its PE matmuls fill gaps in the routing units' PE stream.
                The SBUF hop (s0b) frees the shared xp PSUM slot immediately
                instead of holding it until unit u's routing starts."""
                segs = _segments(u)
                s0T = qs.tile([16, 128], F32, tag="psv")
                for (a, p0, ro, cnt) in segs:
                    for ch in range(CH):
                        nc.tensor.matmul(
                            s0T[:, ro:ro + cnt],
                            wde[:, a, ch, :],
                            xwt[:, ch, p0:p0 + cnt],
                            start=(ch == 0), stop=(ch == CH - 1))
                yield
                s0s = sp.tile([16, 128], F16, tag="s0s")
                nc.scalar.activation(s0s[:], s0T[:], AF.Copy, scale=1.0 / N)
                yield
                s0p = qx.tile([128, 16], F16, tag="xp")
                nc.tensor.transpose(s0p[:], s0s[:], ident[0:16, 0:16])
                yield
                s0b = sp.tile([128, D], F16, tag="s0b")
                nc.scalar.activation(s0b[:], s0p[:], AF.Copy)
                s0_done[u] = s0b
                yield

            def routing_gen(u):
                segs = _segments(u)
                slab = _slab_slot(u)
                bu = bunit[:, u, :]

                s0b = s0_done.pop(u)
                # iter 0: s = s0 + bias
                s = sp.tile([128, D], F32, tag="s0")
                nc.vector.scalar_tensor_tensor(
                    out=s[:], in0=s0b[:], scalar=1.0, in1=bu,
                    op0=AL.mult, op1=AL.add)
                sn, r = squash_pre(s, "0")
                yield
                rt = squash_act(sn, "0")
                yield
                ov = squash_post(s, rt, r, "0", F16)

                lg_prev = None
                for it in (1, 2):
                    sq = str(it)
                    # ---- g-side: vT -> VW on PE -> mul+tree on DVE
                    vTp = qx.tile([16, 128], F16, tag="xp")
                    nc.tensor.transpose(vTp[:], ov[:], ident[:])
                    yield
                    vTs = sp.tile([16, 128], F16, tag="vts")
                    nc.scalar.activation(vTs[:], vTp[:], AF.Copy)
                    yield
                    # VW halves pipelined: the g-side mul+tree for half h
                    # runs on DVE while half h+1 is still on PE/ACT.
                    vws = tp.tile([128, CH, G, Cin], F16, tag="vws")
                    vwsf = vws[:].rearrange("p c g i -> p (c g i)")
                    gv = tp.tile([128, CH, G, Cin], F16, tag="gv")
                    gt = tp.tile([128, CH, G, Cin // 2], F16, tag="gt")
                    lg = lp.tile([128, N], F16, tag="lg" + sq)
                    lgv = lg[:].rearrange("p (c g) -> p c g", g=G)
                    lpv = (None if lg_prev is None else
                           lg_prev[:].rearrange("p (c g) -> p c g", g=G))
                    for h in range(2):
                        vw = qv.tile([128, HW_], F32, tag="vwh")
                        for (a, p0, ro, cnt) in segs:
                            for c0 in (0, 512, 1024):
                                cw = min(512, HW_ - c0)
                                nc.tensor.matmul(
                                    vw[ro:ro + cnt, c0:c0 + cw],
                                    vTs[:, ro:ro + cnt],
                                    wgs[:, a, h * HW_ + c0:h * HW_ + c0 + cw],
                                    start=True, stop=True)
                        yield
                        nc.scalar.activation(
                            vwsf[:, h * HW_:(h + 1) * HW_], vw[:], AF.Copy)
                        yield
                        hs = slice(h * HC, (h + 1) * HC)
                        nc.vector.tensor_mul(gv[:, hs], xws2[:, slab, hs],
                                             vws[:, hs])
                        nc.vector.tensor_add(gt[:, hs], gv[:, hs, :, 0:4],
                                             gv[:, hs, :, 4:8])
                        nc.vector.tensor_add(gv[:, hs, :, 0:2],
                                             gt[:, hs, :, 0:2],
                                             gt[:, hs, :, 2:4])
                        if lg_prev is None:
                            nc.vector.tensor_add(lgv[:, hs], gv[:, hs, :, 0],
                                                 gv[:, hs, :, 1])
                        else:
                            nc.vector.tensor_add(gt[:, hs, :, 0],
                                                 gv[:, hs, :, 0],
                                                 gv[:, hs, :, 1])
                            nc.vector.tensor_add(lgv[:, hs], gt[:, hs, :, 0],
                                                 lpv[:, hs])
                    lg_prev = lg
                    nmx = sp.tile([128, 1], F32, tag="nmx")
                    nc.vector.tensor_reduce(out=nmx[:], in_=lg[:], axis=AX.X,
                                            op=AL.max, negate=True)
                    yield
                    e = sp.tile([128, N], F16, tag="e")
                    se = sp.tile([128, 1], F32, tag="se")
                    nc.scalar.activation(e[:], lg[:], AF.Exp, bias=nmx[:],
                                         scale=1.0, accum_out=se[:])
                    yield
                    rc = sp.tile([128, 1], F32, tag="rc")
                    nc.vector.reciprocal(rc[:], se[:])
                    # ---- s-side: 3 e-transposes -> 18 replication matmuls
                    eTp = qx.tile([96, 3, 128], F16, tag="xp")
                    for k in range(3):
                        nc.tensor.transpose(eTp[:, k, :],
                                            e[:, 96 * k:96 * (k + 1)],
                                            ident[:])
                    yield
                    eTs = sp.tile([96, 3, 128], F16, tag="ets")
                    nc.scalar.activation(eTs[:], eTp[:], AF.Copy)
                    yield
                    # EB halves pipelined: EXT-mul and the first 9 s-matmul
                    # chunks run while the second half is still on PE/ACT.
                    ebs = tp.tile([128, CH, 128], F16, tag="ebs")
                    ebsf = ebs[:].rearrange("q c x -> q (c x)")
                    ext = tp.tile([128, CH, 128], F16, tag="ext")
                    psv = qs.tile([128, 32, D], F32, tag="psv")
                    for h in range(2):
                        eb = qe.tile([128, HC, 128], F32, tag="ebh")
                        for cl in range(HC):
                            ch = h * HC + cl
                            nc.tensor.matmul(
                                eb[:, cl, :],
                                repg[:, ch % 6, :],
                                eTs[:, ch // 6, :],
                                start=True, stop=True)
                        yield
                        nc.scalar.activation(
                            ebsf[:, h * HW_:(h + 1) * HW_],
                            eb[:].rearrange("q c x -> q (c x)"), AF.Copy)
                        yield
                        hs = slice(h * HC, (h + 1) * HC)
                        for (a, p0, ro, cnt) in segs:
                            nc.vector.tensor_mul(
                                ext[:, hs, ro:ro + cnt],
                                xwt[:, hs, p0:p0 + cnt],
                                ebs[:, hs, ro:ro + cnt])
                        for (a, p0, ro, cnt) in segs:
                            for cl in range(HC):
                                ch = h * HC + cl
                                nc.tensor.matmul(
                                    psv[ro:ro + cnt, it, :],
                                    ext[:, ch, ro:ro + cnt],
                                    wde[:, a, ch, :],
                                    start=(ch == 0), stop=(ch == CH - 1))
                    yield
                    s = sp.tile([128, D], F32, tag="s" + sq)
                    nc.vector.scalar_tensor_tensor(
                        out=s[:], in0=psv[:, it, :], scalar=rc[:], in1=bu,
                        op0=AL.mult, op1=AL.add)
                    sn, r = squash_pre(s, sq)
                    yield
                    rt = squash_act(sn, sq)
                    yield
                    ov = squash_post(s, rt, r, sq, F16 if it == 1 else F32)

                for (a, p0, ro, cnt) in segs:
                    nc.sync.dma_start(out_d[a, p0:p0 + cnt, :],
                                      ov[ro:ro + cnt, :])
                yield

            def drain(gens):
                alive = list(gens)
                while alive:
                    nxt = []
                    for g in alive:
                        try:
                            next(g)
                            nxt.append(g)
                        except StopIteration:
                            pass
                    alive = nxt

            drain([s0_gen(0), s0_gen(1), s0_gen(2)])
            for j in range(0, NU, 3):
                g0 = routing_gen(j)
                g1 = routing_gen(j + 1)
                g2 = routing_gen(j + 2)
                for _ in range(4):
                    next(g0)
                next(g1)
                next(g1)
                gens = [g0, g1, g2]
                for kk in (j + 3, j + 4, j + 5):
                    if kk < NU:
                        gens.append(s0_gen(kk))
                drain(gens)

    nc.finalize()
    _prog_cache[key] = nc
    return nc


def _host_prep(x, route_weights, bias):
    x = np.ascontiguousarray(x, dtype=np.float32)
    Wfull = np.ascontiguousarray(route_weights, dtype=np.float32)
    bias = np.ascontiguousarray(bias, dtype=np.float32)

    # im2col: xw[p, n, c], node ordering (ci, ki, kj) as in torch .view
    xw = np.empty((B, w, w, Ci, K, K, Cin), np.float32)
    for ki in range(K):
        for kj in range(K):
            xw[:, :, :, :, ki, kj, :] = (
                x[:, :, ki:ki + w, kj:kj + w, :].transpose(0, 2, 3, 1, 4))
    xw = xw.reshape(P, N, Cin)

    xw4 = xw.reshape(P, CH, G, Cin)
    xwt_h = np.ascontiguousarray(
        xw4.transpose(2, 3, 1, 0).reshape(128, CH, P)).astype(np.float16)

    rows = np.arange(128)
    xws2_h = np.zeros((128, 9, CH, G, Cin), np.float16)
    for q in range(4):
        xws2_h[:, q] = xw4[128 * q + rows]
    for kk in range(4):
        xws2_h[:, 4 + kk] = xw4[64 + 128 * kk + rows]
    xws2_h[:, 8] = xw4[(512 + rows) % P]

    Wn = Wfull.reshape(A, CH, G, Cin, D)
    wgd_h = np.ascontiguousarray(
        Wn.transpose(4, 0, 1, 2, 3).reshape(D, A, CH * G * Cin)
    ).astype(np.float16)
    wde_h = np.ascontiguousarray(
        Wn.transpose(2, 3, 0, 1, 4).reshape(128, A, CH, D)).astype(np.float16)

    repg_h = np.zeros((96, 6, 128), np.float16)
    for j in range(6):
        for g in range(G):
            for c in range(Cin):
                repg_h[j * 16 + g, j, g * 8 + c] = 1.0

    in_maps = []
    for k in range(8):
        a0 = k * AA
        bunit_h = np.empty((128, NU, D), np.float32)
        for u in range(NU):
            rr = np.arange(u * 128, u * 128 + 128)
            bunit_h[:, u, :] = bias[a0 + rr // P]
        im = {
            "xwt": xwt_h,
            "xws2": xws2_h,
            "wgd": np.ascontiguousarray(wgd_h[:, a0:a0 + AA]),
            "wde": np.ascontiguousarray(wde_h[:, a0:a0 + AA]),
            "repg": repg_h,
            "bunit": bunit_h,
        }
        in_maps.append(im)
    return in_maps


def kernel(x, route_weights, bias):
    global LAST_RESULT
    nc = _build_program()
    in_maps = _host_prep(x, route_weights, bias)
    trace = bool(os.environ.get("KERNEL_TRACE"))
    res = run_bass_kernel_spmd(nc, in_maps, list(range(8)), trace=trace)
    LAST_RESULT = res
    full = np.stack([res.results[k]["out"] for k in range(8)])  # [8, AA, P, D]
    full = full.reshape(A, B, w, w, D)
    return np.ascontiguousarray(full.transpose(1, 0, 2, 3, 4))
